# revision 16
# baseline (speedup 1.0000x reference)
"""3-layer GCN (DiffPool-style conv stack) on Trainium2, 8 NeuronCores.

Strategy (graph/data parallel, per sharding hint):
  - Nodes are permuted by degree and dealt round-robin to 8 cores
    (12544 local nodes each incl. dummy padding; 98 blocks of 128).
  - Edges partitioned by destination owner; per core the edge stream is
    grouped by (src quadrant, dst block) so dma_gather indices fit int16
    and each 128-edge tile maps to a single static PSUM block.
  - Per layer: each core computes the table rows for its own nodes
    T = dinv * (H @ W) (node-major), AllGather assembles the full table,
    then per-edge rows are fetched with gpsimd dma_gather (<=1024
    idxs/call, 4 SWDGE queues) and aggregated with one-hot selection
    matmuls into per-block PSUM, accumulated across quadrants in an SBUF
    slab, then scaled/biased/relu'd into the next layer's input.
"""

import sys
import types

sys.path.insert(0, "/opt/trn_rl_repo")

import numpy as np

N = 100000
C = 128
NC = 8
L = 12544           # local nodes per core (98 blocks of 128)
B = L // 128        # 98
NPAD = NC * L       # 100352
QUADS = 4
QROWS = NPAD // QUADS   # 25088 (< 32767, fits int16 gather index)
CALL_MAX_TILES = 8      # dma_gather hardware limit: 1024 indices per call
N_QUEUES = 4

TBL_NP = np.float32      # table dtype; np.float32 or ml_dtypes.bfloat16


def _install_axon_profile_hook():
    """run_bass_kernel_spmd(trace=True) needs antenv.axon_hooks, absent in
    this image; register the equivalent ctypes hook."""
    try:
        import antenv
        if getattr(antenv, "axon_hooks", None) is not None:
            return
        from trn_agent_boot.trn_boot import _ntff_profile_via_ctypes
        mod = types.ModuleType("antenv.axon_hooks")
        hook = _ntff_profile_via_ctypes("/opt/axon/libaxon_pjrt.so")
        mod.get_axon_ntff_profile_hook = lambda: hook
        mod.set_axon_ntff_profile_hook = lambda h: None
        sys.modules["antenv.axon_hooks"] = mod
        antenv.axon_hooks = mod
    except Exception:
        pass


# ----------------------------------------------------------------------------
# Host preprocessing
# ----------------------------------------------------------------------------

def preprocess(x, edge_index):
    """Build the static SPMD schedule + per-core input arrays."""
    x = np.asarray(x, np.float32)
    ei = np.asarray(edge_index, np.int64)
    src = np.concatenate([ei[0], np.arange(N, dtype=np.int64)])
    dst = np.concatenate([ei[1], np.arange(N, dtype=np.int64)])

    deg = np.bincount(dst, minlength=N).astype(np.float32)   # >= 1 (self loops)
    dinv = (1.0 / np.sqrt(deg)).astype(np.float32)

    order = np.argsort(deg, kind="stable")
    rank = np.empty(N, np.int64)
    rank[order] = np.arange(N)
    core_of = rank % NC
    slot_of = rank // NC
    gnew = core_of * L + slot_of

    # original node for (core, slot); -1 for dummy slots
    node_at = -np.ones((NC, L), np.int64)
    node_at[core_of, slot_of] = np.arange(N)

    gsrc = gnew[src]
    gdst = gnew[dst]
    owner = gdst // L
    ldst = gdst % L
    quad = gsrc // QROWS
    qidx = gsrc % QROWS
    blk = ldst // 128
    sid = ldst % 128

    # segment counts per (core, quad, block)
    key = (owner * QUADS + quad) * B + blk
    cnt = np.bincount(key, minlength=NC * QUADS * B).reshape(NC, QUADS, B)
    T = ((cnt + 127) // 128).max(axis=0)          # [QUADS, B] tiles per segment

    # tile schedule: quad-major, block-minor
    tile_q, tile_b = [], []
    seg_tile0 = np.zeros((QUADS, B), np.int64)
    t = 0
    for q in range(QUADS):
        for b in range(B):
            seg_tile0[q, b] = t
            tile_q.extend([q] * int(T[q, b]))
            tile_b.extend([b] * int(T[q, b]))
            t += int(T[q, b])
    tile_q = np.array(tile_q, np.int64)
    tile_b = np.array(tile_b, np.int64)
    n_tiles = t
    S = n_tiles * 128

    # calls: each segment split into <=CALL_MAX_TILES-tile calls; the last
    # call of a segment carries that segment's trailing -1 pads.
    calls = []   # (q, tile0, ntiles)
    for q in range(QUADS):
        for b in range(B):
            tqb = int(T[q, b])
            if tqb == 0:
                continue
            off = 0
            while off < tqb:
                n = min(CALL_MAX_TILES, tqb - off)
                calls.append((q, int(seg_tile0[q, b]) + off, n))
                off += n
    n_calls = len(calls)

    # per-block quad participation (static)
    quads_of_b = [[q for q in range(QUADS) if T[q, b] > 0] for b in range(B)]

    # per-core slot arrays; pad slots gather a valid (spread) row but carry
    # sid=-999 so their one-hot column is all zeros. Spread rows avoid HBM
    # hot-row contention and keep every gather tile fully written (needed
    # for both HW determinism and the simulator's ownership model).
    pad_rows = (np.arange(S, dtype=np.int64) * 97) % QROWS
    idx16 = np.tile(pad_rows.astype(np.int16)[None, :], (NC, 1))
    sidf = np.full((NC, S), -999.0, np.float32)

    eorder = np.lexsort((qidx, blk, quad, owner))
    so, sq, sb_, sqi, ssid = (owner[eorder], quad[eorder], blk[eorder],
                              qidx[eorder], sid[eorder])
    skey = key[eorder]
    # within-group rank
    grp_change = np.flatnonzero(np.diff(skey, prepend=-1))
    grp_id = np.cumsum(np.isin(np.arange(len(skey)), grp_change))
    grp_starts = np.zeros(len(skey), np.int64)
    grp_starts[grp_change] = np.arange(len(skey))[grp_change]
    np.maximum.accumulate(grp_starts, out=grp_starts)
    ranks = np.arange(len(skey)) - grp_starts

    slot = seg_tile0[sq, sb_] * 128 + ranks
    idx16[so, slot] = sqi.astype(np.int16)
    sidf[so, slot] = ssid.astype(np.float32)

    callcnt = np.tile(np.array([n * 128 for (_, _, n) in calls],
                               np.int32)[None, :], (NC, 1))

    # wrapped per-core arrays
    idx_wr = np.zeros((NC, 128, S // 16), np.int16)
    sid_wr = np.zeros((NC, 128, S // 128), np.float32)
    for k in range(NC):
        w16 = idx16[k].reshape(S // 16, 16).T            # [16, S/16]
        idx_wr[k] = np.tile(w16, (8, 1))
        sid_wr[k] = sidf[k].reshape(S // 128, 128).T     # [128, S/128]

    # per-core node-major inputs
    xT = np.zeros((NC, 128, L), np.float32)
    dinv_wr = np.zeros((NC, 128, B), np.float32)
    for k in range(NC):
        nodes = node_at[k]
        real = nodes >= 0
        xk = np.zeros((L, C), np.float32)
        xk[real] = x[nodes[real]]
        xT[k] = xk.T
        dk = np.zeros(L, np.float32)
        dk[real] = dinv[nodes[real]]
        dinv_wr[k] = dk.reshape(B, 128).T

    return dict(
        node_at=node_at, dinv=dinv, T=T, S=S, n_tiles=n_tiles,
        tile_q=tile_q, tile_b=tile_b, seg_tile0=seg_tile0,
        calls=calls, n_calls=n_calls, quads_of_b=quads_of_b,
        idx16=idx16, sidf=sidf, callcnt=callcnt,
        idx_wr=idx_wr, sid_wr=sid_wr, xT=xT, dinv_wr=dinv_wr,
    )


def numpy_model(prep, x, Ws, bs, tbl_dt=None):
    """Exact numpy emulation of the device algorithm (for validation)."""
    if tbl_dt is None:
        tbl_dt = TBL_NP
    node_at = prep["node_at"]
    dinv_wr = prep["dinv_wr"]

    # dinv per (core, local) in node-major
    dloc = np.stack([dinv_wr[k].T.reshape(L) for k in range(NC)])   # [NC, L]
    H = np.stack([prep["xT"][k].T for k in range(NC)])              # [NC, L, C]

    out = None
    for l in range(3):
        # table build
        shards = []
        for k in range(NC):
            tk = (H[k].astype(np.float32) @ Ws[l]) * dloc[k][:, None]
            shards.append(tk.astype(tbl_dt))
        table = np.concatenate(shards, axis=0)      # [NPAD, C]

        # aggregation
        Hn = np.zeros((NC, L, C), np.float32)
        for k in range(NC):
            idx = prep["idx16"][k]
            sidf = prep["sidf"][k]
            S_acc = np.zeros((L, C), np.float32)
            valid = sidf >= 0
            tq = np.repeat(prep["tile_q"], 128)
            tb = np.repeat(prep["tile_b"], 128)
            rows = (prep["idx16"][k][valid].astype(np.int64)
                    + tq[valid] * QROWS)
            tgt = tb[valid] * 128 + sidf[valid].astype(np.int64)
            np.add.at(S_acc, tgt, table[rows].astype(np.float32))
            z = S_acc * dloc[k][:, None] + bs[l][None, :]
            Hn[k] = np.maximum(z, 0.0)
        H = Hn
        out = H
    # assemble
    full = np.zeros((N, C), np.float32)
    for k in range(NC):
        real = node_at[k] >= 0
        full[node_at[k][real]] = out[k][real]
    return full


# ----------------------------------------------------------------------------
# Bass program
# ----------------------------------------------------------------------------

def build_nc(prep, tbl_dt_np=None, debug_stage=None):
    import concourse.bass as bass
    import concourse.mybir as mybir
    import concourse.tile as tile
    from concourse import bacc

    if tbl_dt_np is None:
        tbl_dt_np = TBL_NP
    TBL_DT = mybir.dt.from_np(np.dtype(tbl_dt_np))
    F32 = mybir.dt.float32

    S = prep["S"]
    n_tiles = prep["n_tiles"]
    calls = prep["calls"]
    n_calls = prep["n_calls"]
    tile_q = prep["tile_q"]
    tile_b = prep["tile_b"]
    T = prep["T"]
    seg_tile0 = prep["seg_tile0"]
    quads_of_b = prep["quads_of_b"]

    nc = bacc.Bacc("TRN2", target_bir_lowering=False, debug=False,
                   num_devices=NC, num_swdge_queues=N_QUEUES)

    # inputs
    xT_in = nc.dram_tensor("xT", [128, L], F32, kind="ExternalInput")
    w_in = [nc.dram_tensor(f"W{i+1}", [128, 128], F32, kind="ExternalInput")
            for i in range(3)]
    bias_in = [nc.dram_tensor(f"Bt{i+1}", [128, 128], F32, kind="ExternalInput")
               for i in range(3)]
    iota_in = nc.dram_tensor("iota", [128, 128], F32, kind="ExternalInput")
    ident_in = nc.dram_tensor("ident", [128, 128], F32, kind="ExternalInput")
    dinv_in = nc.dram_tensor("dinv", [128, B], F32, kind="ExternalInput")
    sid_in = nc.dram_tensor("sid", [128, S // 128], F32, kind="ExternalInput")
    idx_in = nc.dram_tensor("idx", [128, S // 16], mybir.dt.int16,
                            kind="ExternalInput")
    out_dram = nc.dram_tensor("out", [L, 128], F32, kind="ExternalOutput")
    tbl_dbg_in = None
    slab_dbg = None
    if debug_stage == "agg_only":
        tbl_dbg_in = nc.dram_tensor("tbl_dbg", [NPAD, 128], TBL_DT,
                                    kind="ExternalInput")
        slab_dbg = nc.dram_tensor("slab_dbg", [L, 128], F32,
                                  kind="ExternalOutput")
        g_dbg = nc.dram_tensor("g_dbg", [128, CALL_MAX_TILES * 128], TBL_DT,
                               kind="ExternalOutput")
        a_dbg = nc.dram_tensor("a_dbg", [128, 128], F32,
                               kind="ExternalOutput")

    from contextlib import ExitStack

    with tile.TileContext(nc) as tc, ExitStack() as es:
        constp = es.enter_context(tc.tile_pool(name="const", bufs=1))
        idxp = es.enter_context(tc.tile_pool(name="idxp", bufs=1))
        xtp = es.enter_context(tc.tile_pool(name="xt", bufs=3))
        gatp = es.enter_context(tc.tile_pool(name="gat", bufs=8))
        app = es.enter_context(tc.tile_pool(name="ap", bufs=4))
        slabp = es.enter_context(tc.tile_pool(name="slab", bufs=B))
        workp = es.enter_context(tc.tile_pool(name="work", bufs=4))
        tblp = es.enter_context(tc.tile_pool(name="tblp", bufs=3))
        htp = es.enter_context(tc.tile_pool(name="htp", bufs=2))
        aggps = es.enter_context(tc.tile_pool(name="aggps", bufs=2, space="PSUM"))
        tpps = es.enter_context(tc.tile_pool(name="tpps", bufs=2, space="PSUM"))
        gemmps = es.enter_context(tc.tile_pool(name="gemmps", bufs=2, space="PSUM"))
        dramp = es.enter_context(tc.tile_pool(name="dram", bufs=1, space="DRAM"))
        if True:

            # ---- resident constants ----
            w_sb = []
            bias_sb = []
            for i in range(3):
                w = constp.tile([128, 128], F32, tag=f"w{i}")
                nc.sync.dma_start(w[:], w_in[i][:, :])
                w_sb.append(w)
                bb = constp.tile([128, 128], F32, tag=f"bias{i}")
                nc.sync.dma_start(bb[:], bias_in[i][:, :])
                bias_sb.append(bb)
            iota_sb = constp.tile([128, 128], F32, tag="iota")
            nc.sync.dma_start(iota_sb[:], iota_in[:, :])
            ident_sb = constp.tile([128, 128], F32, tag="ident")
            nc.sync.dma_start(ident_sb[:], ident_in[:, :])
            dinv_sb = constp.tile([128, B], F32, tag="dinv")
            nc.sync.dma_start(dinv_sb[:], dinv_in[:, :])
            sid_sb = constp.tile([128, S // 128], F32, tag="sid")
            nc.sync.dma_start(sid_sb[:], sid_in[:, :])
            idx_sb = idxp.tile([128, S // 16], mybir.dt.int16, tag="idx")
            nc.sync.dma_start(idx_sb[:], idx_in[:, :])

            myshard = dramp.tile([L, 128], TBL_DT, tag="myshard")
            table = dramp.tile([NPAD, 128], TBL_DT, tag="table")

            def do_allgather():
                nc.gpsimd.collective_compute(
                    "AllGather",
                    mybir.AluOpType.bypass,
                    replica_groups=[list(range(NC))],
                    ins=[myshard.opt()],
                    outs=[table.opt()],
                )

            def table_row_block(l, b, lhsT_sb):
                """GEMM + dinv scale + store to myshard rows of block b."""
                ps = gemmps.tile([128, 128], F32, tag="gemm")
                nc.tensor.matmul(ps[:], lhsT=lhsT_sb[:], rhs=w_sb[l][:],
                                 start=True, stop=True)
                tb = tblp.tile([128, 128], TBL_DT, tag="tbl")
                nc.vector.tensor_scalar(tb[:], ps[:], dinv_sb[:, b:b + 1],
                                        None, op0=mybir.AluOpType.mult)
                nc.sync.dma_start(myshard[b * 128:(b + 1) * 128, :], tb[:])

            # ---- phase A: layer-1 table from x ----
            if debug_stage == "agg_only":
                nc.sync.dma_start(table[:, :], tbl_dbg_in[:, :])
            else:
                for b in range(B):
                    xt = xtp.tile([128, 128], F32, tag="xt")
                    nc.sync.dma_start(xt[:], xT_in[:, b * 128:(b + 1) * 128])
                    table_row_block(0, b, xt)
                if debug_stage != "phaseA":
                    do_allgather()

            # ---- layers ----
            if debug_stage in ("phaseA", "table1"):
                n_layers = 0
            elif debug_stage in ("layer1", "agg_only"):
                n_layers = 1
            else:
                n_layers = 3
            for l in range(n_layers):
                slabs = [None] * B
                psq = {}

                def block_tail(b):
                    s = slabs[b]
                    if slab_dbg is not None:
                        nc.sync.dma_start(
                            slab_dbg[b * 128:(b + 1) * 128, :], s[:])
                    u = workp.tile([128, 128], F32, tag="u")
                    nc.vector.scalar_tensor_tensor(
                        u[:], s[:], dinv_sb[:, b:b + 1], bias_sb[l][:],
                        op0=mybir.AluOpType.mult, op1=mybir.AluOpType.add)
                    h = workp.tile([128, 128], F32, tag="h")
                    nc.vector.tensor_scalar(h[:], u[:], 0.0, None,
                                            op0=mybir.AluOpType.max)
                    if l == 2:
                        nc.sync.dma_start(out_dram[b * 128:(b + 1) * 128, :],
                                          h[:])
                        return
                    tp = tpps.tile([128, 128], F32, tag="tp")
                    nc.tensor.transpose(tp[:], h[:], ident_sb[:])
                    htt = htp.tile([128, 128], F32, tag="ht")
                    nc.vector.tensor_copy(htt[:], tp[:])
                    table_row_block(l + 1, b, htt)

                for ci, (q, t0, ntl) in enumerate(calls):
                    g = gatp.tile([128, CALL_MAX_TILES, 128], TBL_DT, tag="g")
                    nc.gpsimd.dma_gather(
                        g[:, 0:ntl, :],
                        table[q * QROWS:(q + 1) * QROWS, :],
                        idx_sb[:, t0 * 8:(t0 + ntl) * 8],
                        ntl * 128, ntl * 128, 128,
                        queue_num=ci % N_QUEUES,
                    )
                    if ci == 0 and debug_stage == "agg_only":
                        nc.sync.dma_start(
                            g_dbg[:, 0:ntl * 128],
                            g[:, 0:ntl, :].rearrange("p t f -> p (t f)"))
                    for tl in range(ntl):
                        gt = t0 + tl
                        b = int(tile_b[gt])
                        a = app.tile([128, 128], F32, tag="a")
                        nc.vector.tensor_scalar(
                            a[:], iota_sb[:], sid_sb[:, gt:gt + 1], None,
                            op0=mybir.AluOpType.is_equal)
                        if ci == 0 and tl == 0 and debug_stage == "agg_only":
                            nc.sync.dma_start(a_dbg[:, :], a[:])
                        first = (gt == seg_tile0[q, b])
                        last = (gt == seg_tile0[q, b] + T[q, b] - 1)
                        if first:
                            psq[b] = aggps.tile([128, 128], F32, tag="agg", name=f"agg_{l}_{q}_{b}")
                        nc.tensor.matmul(psq[b][:], lhsT=a[:],
                                         rhs=g[:, tl, :],
                                         start=first, stop=last)
                        if last:
                            qs = quads_of_b[b]
                            if q == qs[0]:
                                slabs[b] = slabp.tile([128, 128], F32, tag="slab", name=f"slab_{l}_{b}")
                                nc.vector.tensor_copy(slabs[b][:], psq[b][:])
                            else:
                                nc.vector.tensor_tensor(
                                    slabs[b][:], slabs[b][:], psq[b][:],
                                    op=mybir.AluOpType.add)
                            if q == qs[-1]:
                                block_tail(b)
                if l < 2 and debug_stage != "agg_only":
                    do_allgather()

    nc.compile()
    return nc


# ----------------------------------------------------------------------------
# Runner
# ----------------------------------------------------------------------------

def make_in_maps(prep, Ws, bs):
    iota = np.tile(np.arange(128, dtype=np.float32)[None, :], (128, 1))
    ident = np.eye(128, dtype=np.float32)
    maps = []
    for k in range(NC):
        maps.append({
            "xT": prep["xT"][k],
            "W1": Ws[0].astype(np.float32),
            "W2": Ws[1].astype(np.float32),
            "W3": Ws[2].astype(np.float32),
            "Bt1": np.tile(bs[0][None, :], (128, 1)).astype(np.float32),
            "Bt2": np.tile(bs[1][None, :], (128, 1)).astype(np.float32),
            "Bt3": np.tile(bs[2][None, :], (128, 1)).astype(np.float32),
            "iota": iota,
            "ident": ident,
            "dinv": prep["dinv_wr"][k],
            "sid": prep["sid_wr"][k],
            "idx": prep["idx_wr"][k],
        })
    return maps


def assemble_output(prep, results):
    full = np.zeros((N, C), np.float32)
    for k in range(NC):
        nodes = prep["node_at"][k]
        real = nodes >= 0
        full[nodes[real]] = results[k]["out"][real]
    return full


_CACHE = {}


def run(inputs, trace=False, sim=False):
    from concourse.bass_utils import run_bass_kernel_spmd

    x = np.asarray(inputs["x"], np.float32)
    Ws = [np.asarray(inputs[f"W{i+1}"], np.float32) for i in range(3)]
    bs = [np.asarray(inputs[f"b{i+1}"], np.float32) for i in range(3)]

    prep = preprocess(x, inputs["edge_index"])
    ckey = ("nc", TBL_NP, prep["S"], prep["n_calls"])
    if ckey not in _CACHE:
        _CACHE[ckey] = build_nc(prep)
    nc = _CACHE[ckey]

    in_maps = make_in_maps(prep, Ws, bs)

    if sim:
        from concourse.bass_interp import MultiCoreSim
        msim = MultiCoreSim(nc, NC, trace=False, require_finite=False,
                            require_nnan=False)
        for k in range(NC):
            for name, arr in in_maps[k].items():
                msim.cores[k].tensor(name)[:] = arr
        msim.simulate(check_with_hw=False)
        results = [{"out": np.array(msim.cores[k].tensor("out"))}
                   for k in range(NC)]
        return assemble_output(prep, results), None

    if trace:
        _install_axon_profile_hook()
    res = run_bass_kernel_spmd(nc, in_maps, list(range(NC)), trace=trace)
    return assemble_output(prep, res.results), res


def kernel(**inputs):
    out, _ = run(inputs)
    return out
